# revision 45
# baseline (speedup 1.0000x reference)
"""Trainium2 Bass kernel for nn_DirModel (quaternion Dirac GNN message passing).

Strategy (8 NeuronCores, B=2 samples):
  - 4 cores per sample: core c owns sample s=c//4 and slice r=c%4 of the face
    rows (Di output) / node rows (DiA output).
  - Di/DiA are host-side transposed/permuted/sliced, cast to fp8(e4m3), and
    kept RESIDENT in SBUF (8+8 MB per core) across all 5 blocks.
  - Big matmuls keep the activations stationary (lhsT [128,16] quaternion
    slices) and stream the resident fp8 matrix as moving operand,
    accumulating over the contraction in PSUM. The 4 output groups run
    column-tiled (tile_position=(0,32j)) with each group in its OWN PSUM
    bank, and the accumulation steps are emitted interleaved with the
    produce chunks (4 steps per chunk) so the PE stream is dense with no
    serial tail.
  - Both samples are stacked on the partition axis: states vTB/fTB are
    [128, rows] bf16 with partitions 64*s+c. All elementwise/BN work runs at
    full 128-lane width; the per-node linears contract K=128 (both samples at
    once) with duplicated weights, the non-owned sample zeroed via msel
    premultiplied into gamma/beta host-side.
  - Per block, slices are exchanged with one 8-rank AllGather per direction.
  - BN batch stats piggyback on the elu ops (accum_out); the cross-partition
    combine uses a selector matmul with 1/T pre-folded so it yields E[x],
    E[x^2] directly; rstd via ACT Sqrt + DVE reciprocal. The per-block bias
    row (with the BN shift folded in) is preloaded into PSUM by 1-2 wide K=1
    matmuls per segment (rhs = host-replicated 512-wide bias row) instead of
    one K=1 matmul per chunk.
"""

import numpy as np
import ml_dtypes

import concourse.bass as bass
import concourse.mybir as mybir
import concourse.tile as tile
from concourse import bacc
from concourse.bass_utils import run_bass_kernel_spmd

B, N, F, C = 2, 1024, 2048, 64
NB = 5
NCORES = 8

F32 = mybir.dt.float32
BF16 = mybir.dt.bfloat16
FP8 = mybir.dt.float8e4
NP_BF16 = ml_dtypes.bfloat16
NP_FP8 = ml_dtypes.float8_e4m3
AF = mybir.ActivationFunctionType
ALU = mybir.AluOpType
RG = [list(range(NCORES))]


def _build():
    nc = bacc.Bacc(
        "TRN2",
        target_bir_lowering=False,
        debug=False,
        enable_asserts=False,
        num_devices=NCORES,
    )

    # ---------------- DRAM I/O ----------------
    dit_d = nc.dram_tensor("dit", [128, 32, 2048], FP8, kind="ExternalInput")
    diat_d = nc.dram_tensor("diat", [128, 64, 1024], FP8, kind="ExternalInput")
    inpT_d = nc.dram_tensor("inpT", [3, B, N], BF16, kind="ExternalInput")
    w_in_d = nc.dram_tensor("w_in", [3, C], BF16, kind="ExternalInput")
    b_in2_d = nc.dram_tensor("b_in2", [128, 1], F32, kind="ExternalInput")
    w0_d = nc.dram_tensor("w0", [128, NB, C], BF16, kind="ExternalInput")
    w1_d = nc.dram_tensor("w1", [128, NB, C], BF16, kind="ExternalInput")
    b0_d = nc.dram_tensor("b0", [1, NB, C], BF16, kind="ExternalInput")
    b1_d = nc.dram_tensor("b1", [1, NB, C], BF16, kind="ExternalInput")
    g0_d = nc.dram_tensor("g0", [128, NB], F32, kind="ExternalInput")
    be0_d = nc.dram_tensor("be0", [128, NB], F32, kind="ExternalInput")
    g1_d = nc.dram_tensor("g1", [128, NB], F32, kind="ExternalInput")
    be1_d = nc.dram_tensor("be1", [128, NB], F32, kind="ExternalInput")
    sselv_d = nc.dram_tensor("sselv", [128, 128], F32, kind="ExternalInput")
    bn2g_d = nc.dram_tensor("bn2g", [128, 1], F32, kind="ExternalInput")
    bn2b_d = nc.dram_tensor("bn2b", [128, 1], F32, kind="ExternalInput")
    w2_d = nc.dram_tensor("w2", [128, C], BF16, kind="ExternalInput")
    b2_d = nc.dram_tensor("b2", [1, C], BF16, kind="ExternalInput")
    maskc_d = nc.dram_tensor("maskc", [128, 8, B], BF16, kind="ExternalInput")
    maskrow_d = nc.dram_tensor("maskrow", [B, N], BF16, kind="ExternalInput")
    wfc_d = nc.dram_tensor("wfc", [C, 10], BF16, kind="ExternalInput")
    bfc_d = nc.dram_tensor("bfc", [B, 10], F32, kind="ExternalInput")
    out_d = nc.dram_tensor("out", [B, 10], F32, kind="ExternalOutput")

    with tile.TileContext(nc) as tc:
        with (
            tc.tile_pool(name="res", bufs=1) as res,
            tc.tile_pool(name="sb", bufs=2) as sb,
            tc.tile_pool(name="sc", bufs=2) as sc,
            tc.tile_pool(name="st", bufs=4) as st,
            tc.tile_pool(name="pacc", bufs=1, space="PSUM") as pacc,
            tc.tile_pool(name="px", bufs=1, space="PSUM") as px,
            tc.tile_pool(name="pm", bufs=1, space="PSUM") as pm,
            tc.tile_pool(name="dram", bufs=2, space="DRAM") as dram,
        ):
            # ------------- resident loads (spread across the 3 DGE queues;
            # dit chunks land first so block-0's Di matmul can ride them) ----
            small_engs = (nc.sync, nc.scalar)
            _sl = [0]

            def load(name, shape, dtype, src):
                t = res.tile(shape, dtype, name=name)
                small_engs[_sl[0] % 2].dma_start(t[:], src.ap())
                _sl[0] += 1
                return t

            inpT_sb = load("inpT_sb", [3, B, N], BF16, inpT_d)
            w_in_sb = load("w_in_sb", [3, C], BF16, w_in_d)
            b_in2_sb = load("b_in2_sb", [128, 1], F32, b_in2_d)
            w0_sb = load("w0_sb", [128, NB, C], BF16, w0_d)
            w1_sb = load("w1_sb", [128, NB, C], BF16, w1_d)
            b0_sb = load("b0_sb", [1, NB, C], BF16, b0_d)
            b1_sb = load("b1_sb", [1, NB, C], BF16, b1_d)
            g0_sb = load("g0_sb", [128, NB], F32, g0_d)
            be0_sb = load("be0_sb", [128, NB], F32, be0_d)
            g1_sb = load("g1_sb", [128, NB], F32, g1_d)
            be1_sb = load("be1_sb", [128, NB], F32, be1_d)
            sselv_sb = load("sselv_sb", [128, 128], F32, sselv_d)
            bn2g_sb = load("bn2g_sb", [128, 1], F32, bn2g_d)
            bn2b_sb = load("bn2b_sb", [128, 1], F32, bn2b_d)
            w2_sb = load("w2_sb", [128, C], BF16, w2_d)
            b2_sb = load("b2_sb", [1, C], BF16, b2_d)
            maskc_sb = load("maskc_sb", [128, 8, B], BF16, maskc_d)
            maskrow_sb = load("maskrow_sb", [B, N], BF16, maskrow_d)
            wfc_sb = load("wfc_sb", [C, 10], BF16, wfc_d)
            bfc_sb = load("bfc_sb", [B, 10], F32, bfc_d)

            # big resident operators: all dit chunks first (block 0 needs
            # them soonest), then diat, round-robin across the 3 DGE queues.
            big_engs = (nc.gpsimd, nc.sync, nc.scalar)
            dit_v = dit_d.ap().rearrange("p (a k) n -> p a k n", a=8)
            dit_cs = []
            for a in range(8):
                t = res.tile([128, 4, 2048], FP8, name=f"dit{a}")
                big_engs[a % 3].dma_start(t[:], dit_v[:, a])
                dit_cs.append(t)
            diat_v = diat_d.ap().rearrange("p (a k) n -> p a k n", a=8)
            diat_cs = []
            for a in range(8):
                t = res.tile([128, 8, 1024], FP8, name=f"diat{a}")
                big_engs[a % 3].dma_start(t[:], diat_v[:, a])
                diat_cs.append(t)

            ones_bf = res.tile([1, 128], BF16)
            nc.vector.memset(ones_bf[:], 1.0)

            # ------------- state: both samples stacked on partitions ------
            vTB = res.tile([128, N], BF16)   # partition 64*s + c
            fTB = res.tile([128, F], BF16)

            # four per-column-group PSUM accumulator banks, shared between
            # the Di (cols 0:512) and DiA (cols 0:256) phases.
            bigp = [pacc.tile([128, 512], F32, tag=f"bg{j}", name=f"bg{j}")
                    for j in range(4)]
            # bias-preloaded produce accumulators; chunks alternate between
            # the two tiles so dependency tracking (tile-granular) doesn't
            # serialize chunk t+1's matmul behind chunk t's elu reads.
            pxt = [px.tile([128, 512], F32, tag=f"px{p}", name=f"px{p}")
                   for p in range(2)]

            # initial v = inputs @ W_in + b_in (borrows pxt)
            for h in range(2):
                for s in range(B):
                    nc.tensor.matmul(
                        pxt[h][64 * s:64 * (s + 1), :],
                        w_in_sb[:],
                        inpT_sb[:, s, 512 * h:512 * (h + 1)],
                        start=True, stop=True,
                        tile_position=(0, 64 * s),
                    )
                nc.vector.tensor_scalar(
                    vTB[:, 512 * h:512 * (h + 1)],
                    pxt[h][:],
                    b_in2_sb[:].opt(), None, ALU.add,
                )
            nc.vector.memset(fTB[:], 0.0)

            def elu_stats(src, R, nm):
                """elu(src)->[128,R] bf16 + per-half stat tiles [128,2]
                (sum, sumsq) so the selector matmul of half 0 can overlap
                half 1's stats."""
                ev = sc.tile([128, R], BF16, tag="evT", name=f"ev{nm}")
                sqd = sc.tile([128, R], BF16, tag="sqT", name=f"sq{nm}")
                stats = []
                H = R // 2
                for h in range(2):
                    cs = slice(H * h, H * (h + 1))
                    e = sc.tile([128, H], BF16, tag=f"eT{h}", name=f"e{nm}{h}")
                    nc.scalar.activation(e[:], src[:, cs], AF.Exp)
                    nc.vector.tensor_scalar(
                        e[:], e[:], -1.0, 0.0, ALU.add, ALU.min)
                    sth = st.tile([128, 2], F32, tag=f"sth{h}",
                                  name=f"sth{nm}{h}")
                    nc.vector.scalar_tensor_tensor(
                        ev[:, cs], src[:, cs], 0.0, e[:], ALU.max, ALU.add,
                        accum_out=sth[:, 0:1],
                    )
                    nc.scalar.activation(
                        sqd[:, cs], ev[:, cs], AF.Square,
                        accum_out=sth[:, 1:2])
                    stats.append(sth)
                return ev, stats

            def rsqrt_chain(pst, g_ap, be_ap, nm):
                """pst [128,2] SBUF = [E[x], E[x^2]] -> (scale f32, shneg
                bf16). Bit-trick + 1 Newton step keeps ACT tables on EXP."""
                m2 = st.tile([128, 1], F32, tag="bns2", name=f"m2{nm}")
                nc.vector.tensor_mul(m2[:], pst[:, 0:1], pst[:, 0:1])
                varp = st.tile([128, 1], F32, tag="bns3", name=f"vp{nm}")
                nc.vector.tensor_tensor(varp[:], pst[:, 1:2], m2[:],
                                        ALU.subtract)
                iv = st.tile([128, 1], mybir.dt.int32, tag="bnsi",
                             name=f"iv{nm}")
                nc.vector.tensor_scalar(
                    iv[:], varp[:].bitcast(mybir.dt.int32), 1, None,
                    ALU.arith_shift_right,
                )
                nc.vector.tensor_scalar(
                    iv[:], iv[:], -1, 0x5F3759DF, ALU.mult, ALU.add
                )
                rstd = st.tile([128, 1], F32, tag="bns4", name=f"rstd{nm}")
                nc.vector.tensor_copy(rstd[:], iv[:].bitcast(F32))
                nt = st.tile([128, 1], F32, tag="bns5", name=f"nt{nm}")
                nc.vector.tensor_mul(nt[:], rstd[:], rstd[:])
                nc.vector.tensor_mul(nt[:], nt[:], varp[:])
                nc.vector.tensor_scalar(
                    nt[:], nt[:], -0.5, 1.5, ALU.mult, ALU.add
                )
                nc.vector.tensor_mul(rstd[:], rstd[:], nt[:])
                scale = st.tile([128, 1], F32, tag="selsc", name=f"sc{nm}")
                nc.vector.tensor_mul(scale[:], rstd[:], g_ap)
                shneg = st.tile([128, 1], BF16, tag="selsh", name=f"sh{nm}")
                nc.vector.scalar_tensor_tensor(
                    shneg[:], pst[:, 0:1], scale[:].opt(), be_ap,
                    ALU.mult, ALU.subtract,
                )
                return scale, shneg

            def bn_fold(ps, stats, ssel, T, g_ap, be_ap, w_ap, b_ap, nm):
                """-> (Wp bf16 [128,C], brow bf16 [1,C]).

                Per-half accumulating selector matmuls (half 0's can run
                while half 1's stats finish). The 1/T scaling happens on
                the DVE during the PSUM->SBUF copy (full f32), not in the
                PE fp32r matmul."""
                for h in range(2):
                    nc.tensor.matmul(ps[:, 0:2], ssel, stats[h][:],
                                     start=(h == 0), stop=(h == 1))
                pst = st.tile([128, 2], F32, tag="pstc", name=f"pst{nm}")
                nc.vector.tensor_scalar(pst[:], ps[:, 0:2], 1.0 / T, None,
                                        ALU.mult)
                scale, shneg = rsqrt_chain(pst, g_ap, be_ap, nm)
                Wp = sc.tile([128, C], BF16, tag="wp", name=f"wp{nm}")
                nc.vector.tensor_scalar(
                    Wp[:], w_ap, scale[:].opt(), None, ALU.mult)
                psr = pm.tile([1, C], F32, tag="prow", name=f"psr{nm}")
                nc.tensor.matmul(psr[:], shneg[:], w_ap, start=True,
                                 stop=True)
                brow = sc.tile([1, C], BF16, tag="brow", name=f"brow{nm}")
                nc.vector.tensor_tensor(brow[:], b_ap, psr[:], ALU.subtract)
                return Wp, brow

            def stage_gather(nsz, nm):
                """CAST the 4 group banks -> compact fp8 stage, 1 DMA out,
                AllGather, and return the SBUF view of the gathered data."""
                stg = sb.tile([128, nsz], FP8, tag="stg", name=f"stg{nm}")
                for j in range(4):
                    if j % 2 == 0:
                        nc.vector.tensor_copy(
                            stg[32 * j:32 * j + 16, :],
                            bigp[j][32 * j:32 * j + 16, 0:nsz])
                    else:
                        nc.scalar.copy(
                            stg[32 * j:32 * j + 16, :],
                            bigp[j][32 * j:32 * j + 16, 0:nsz])
                agin = dram.tile([C, nsz], FP8, tag="agin", name=f"agin{nm}")
                nc.sync.dma_start(
                    agin[:].rearrange("(a b) n -> a b n", a=4),
                    stg[:].rearrange("(a b) n -> a b n", a=4)[:, 0:16, :],
                )
                agout = dram.tile([NCORES * C, nsz], FP8, tag="agout",
                                  name=f"agout{nm}", addr_space="Shared")
                nc.gpsimd.collective_compute(
                    "AllGather", ALU.bypass, replica_groups=RG,
                    ins=[agin.opt()], outs=[agout.opt()],
                )
                agv = agout[:].rearrange("(s r c) n -> s c r n", s=2, r=4)
                gB = sb.tile([128, 4 * nsz], FP8, tag="gB", name=f"gB{nm}")
                in_engs = (nc.sync, nc.scalar)
                # r-pair split so the half-0 state add can start before the
                # second half of the gathered payload has landed.
                g_engs = (nc.sync, nc.scalar, nc.gpsimd, nc.sync)
                for rp in range(2):
                    for s in range(B):
                        g_engs[2 * rp + s].dma_start(
                            gB[C * s:C * (s + 1),
                               2 * rp * nsz:2 * (rp + 1) * nsz].rearrange(
                                "c (r n) -> c r n", r=2),
                            agv[s][:, 2 * rp:2 * (rp + 1), :])
                return gB

            def warm_pe(ps, gB, ncols, nm):
                """~16 tiny matmuls anchored on the gathered payload keep
                the PE's HAM clock un-throttled through the stats phase so
                the real matmul stream starts warm."""
                for k in range(16):
                    nc.tensor.matmul(
                        ps[:, 2:4],
                        gB[:, 128 * (k % (ncols // 128)):
                           128 * (k % (ncols // 128)) + 128],
                        gB[:, 0:2],
                        start=True, stop=True,
                    )

            def accum_state(stT, gB, R):
                """state += gathered, split across DVE / GpSimd halves so the
                elu halves can chain onto their own half's add."""
                H = R // 2
                nc.vector.tensor_tensor(
                    stT[:, 0:H], stT[:, 0:H], gB[:, 0:H], ALU.add)
                nc.vector.tensor_tensor(
                    stT[:, H:R], stT[:, H:R], gB[:, H:R], ALU.add)

            def produce_chunk(ev_ap, t, Wp, out_dtype, nm):
                """row[t] [128, C] = elu(ev[:,128t:].T @ W' + brow).

                Bias was preloaded into pxt by the wide K=1 matmuls."""
                psx = pxt[t % 2][:, 64 * (t // 2):64 * (t // 2) + 64]
                nc.tensor.matmul(psx, ev_ap, Wp[:], start=False, stop=True)
                e2 = sc.tile([128, C], BF16, tag="e2", name=f"e2{nm}{t}")
                nc.scalar.activation(e2[:], psx, AF.Exp)
                nc.vector.tensor_scalar(e2[:], e2[:], -1.0, 0.0,
                                        ALU.add, ALU.min)
                row = sc.tile([128, C], out_dtype, tag=f"rowt{t}",
                              name=f"row{nm}{t}")
                nc.vector.scalar_tensor_tensor(
                    row[:], psx, 0.0, e2[:], ALU.max, ALU.add
                )
                return row

            def preload_bias(brow, nchunks):
                rhs = brow[:].unsqueeze(1).broadcast_to([1, 8, C])
                for h in range(2):
                    nc.tensor.matmul(
                        pxt[h][:], ones_bf[:],
                        rhs, start=True, stop=False,
                    )

            LAG = 2

            def di_steps(t, xr_t, i):
                for jj in range(4):
                    kk = 4 * t + jj
                    for j in range(4):
                        nc.tensor.matmul(
                            bigp[j][32 * j:32 * j + 16, :],
                            xr_t[:, 16 * jj:16 * (jj + 1)],
                            dit_cs[t][:, jj, 512 * j:512 * (j + 1)],
                            start=(kk == 0), stop=(kk == 31),
                            tile_position=(0, 32 * j),
                        )

            def dia_steps(t, yr_t, i):
                for m in range(4):
                    kk = 4 * t + m
                    for j in range(4):
                        nc.tensor.matmul(
                            bigp[j][32 * j:32 * j + 16, 0:256],
                            yr_t[:, 16 * m:16 * (m + 1)],
                            diat_cs[kk // 8][:, kk % 8,
                                             256 * j:256 * (j + 1)],
                            start=(kk == 0), stop=(kk == 63),
                            tile_position=(0, 32 * j),
                        )

            ps_next = None
            for i in range(NB):
                # ======== v side: x = elu(conv(elu(v))), f += Di @ x ======
                ev, vstats = elu_stats(vTB[:], N, f"v{i}")
                if ps_next is None:
                    ps_next = pm.tile([128, 4], F32, tag="pstat",
                                      name=f"psv{i}")
                Wp0, brow0 = bn_fold(
                    ps_next, vstats, sselv_sb[:], float(B * N),
                    g0_sb[:, i:i + 1].opt(),
                    be0_sb[:, i:i + 1].opt(), w0_sb[:, i, :],
                    b0_sb[:, i, :], f"v{i}",
                )
                preload_bias(brow0, 8)
                xrs = {}
                for t in range(8 + LAG):
                    if t < 8:
                        xrs[t] = produce_chunk(
                            ev[:, 128 * t:128 * (t + 1)], t, Wp0, FP8,
                            f"x{i}")
                    if t >= LAG:
                        di_steps(t - LAG, xrs[t - LAG], i)
                gfB = stage_gather(512, f"f{i}")
                psf = pm.tile([128, 4], F32, tag="pstat", name=f"psf{i}")
                warm_pe(psf, gfB, 2048, f"f{i}")
                accum_state(fTB, gfB, F)

                # ======== f side: y = elu(conv(elu(f))), v += DiA @ y =====
                ef, fstats = elu_stats(fTB[:], F, f"f{i}")
                Wp1, brow1 = bn_fold(
                    psf, fstats, sselv_sb[:], float(B * F),
                    g1_sb[:, i:i + 1].opt(),
                    be1_sb[:, i:i + 1].opt(), w1_sb[:, i, :],
                    b1_sb[:, i, :], f"f{i}",
                )
                preload_bias(brow1, 16)
                yrs = {}
                for t in range(16 + LAG):
                    if t < 16:
                        yrs[t] = produce_chunk(
                            ef[:, 128 * t:128 * (t + 1)], t, Wp1, FP8,
                            f"y{i}")
                    if t >= LAG:
                        dia_steps(t - LAG, yrs[t - LAG], i)
                gvB = stage_gather(256, f"v{i}")
                ps_next = pm.tile([128, 4], F32, tag="pstat",
                                  name=f"psv{i + 1}")
                warm_pe(ps_next, gvB, 1024, f"v{i + 1}")
                accum_state(vTB, gvB, N)

            # ======== head (both samples, no selection) ========
            hev, hstats = elu_stats(vTB[:], N, "h")
            ps_h = ps_next
            for h in range(2):
                nc.tensor.matmul(ps_h[:, 0:2], sselv_sb[:], hstats[h][:],
                                 start=(h == 0), stop=(h == 1))
            pst_h = st.tile([128, 2], F32, tag="pstc", name="pst_h")
            nc.vector.tensor_scalar(pst_h[:], ps_h[:, 0:2],
                                    1.0 / float(B * N), None, ALU.mult)
            hscale, hshneg = rsqrt_chain(pst_h, bn2g_sb[:], bn2b_sb[:], "h")
            # head contraction is per-sample (K=64): fold with K=64 shift
            Wp2 = sc.tile([128, C], BF16, tag="wp", name="wp_h")
            nc.vector.tensor_scalar(
                Wp2[:], w2_sb[:], hscale[:].opt(), None, ALU.mult)
            psr_h = pm.tile([1, C], F32, tag="prow", name="psr_h")
            nc.tensor.matmul(psr_h[:], hshneg[0:C, :], w2_sb[0:C, :],
                             start=True, stop=True)
            brow_h = sc.tile([1, C], BF16, tag="brow", name="brow_h")
            nc.vector.tensor_tensor(brow_h[:], b2_sb[:], psr_h[:],
                                    ALU.subtract)
            brow_h_b = brow_h[:].unsqueeze(1).broadcast_to([1, 8, C])
            pooled = sb.tile([C, B], BF16, tag="pooled")
            for hb in range(2):
                nc.tensor.matmul(
                    pxt[hb][:], ones_bf[:], brow_h_b,
                    start=True, stop=False,
                )
            pp_both = pm.tile([C, B], F32, tag="prow", name="pp_both")
            hrows = {}
            for g in range(16 + LAG):
                if g < 16:
                    s, t = divmod(g, 8)
                    psx = pxt[g % 2][:, 64 * (g // 2):64 * (g // 2) + 64]
                    nc.tensor.matmul(
                        psx, hev[64 * s:64 * (s + 1),
                                 128 * t:128 * (t + 1)],
                        Wp2[64 * s:64 * (s + 1), :],
                        start=False, stop=True,
                    )
                    e2 = sc.tile([128, C], BF16, tag="e2", name=f"e2h{g}")
                    nc.scalar.activation(e2[:], psx, AF.Exp)
                    nc.vector.tensor_scalar(e2[:], e2[:], -1.0, 0.0,
                                            ALU.add, ALU.min)
                    row = sc.tile([128, C], BF16, tag=f"rowt{g % 8}",
                                  name=f"rowh{g}")
                    nc.vector.scalar_tensor_tensor(
                        row[:], psx, 0.0, e2[:], ALU.max, ALU.add
                    )
                    hrows[g] = row
                if g >= LAG:
                    s2, t2 = divmod(g - LAG, 8)
                    nc.tensor.matmul(
                        pp_both[:, s2:s2 + 1], hrows[g - LAG][:],
                        maskc_sb[:, t2, s2:s2 + 1],
                        start=(t2 == 0), stop=(t2 == 7),
                    )
            nc.vector.tensor_copy(pooled[:], pp_both[:])
            msum = st.tile([B, 1], F32, tag="hd", name="msum")
            nc.vector.tensor_reduce(
                msum[:], maskrow_sb[:], mybir.AxisListType.X, ALU.add
            )
            rec = st.tile([B, 1], F32, tag="hd", name="rec")
            nc.vector.reciprocal(rec[:], msum[:])
            pl = pm.tile([B, 10], F32, tag="pstat", name="pl")
            nc.tensor.matmul(pl[:], pooled[:], wfc_sb[:], start=True,
                             stop=True)
            lu = sb.tile([B, 10], F32, tag="hd2", name="lu")
            nc.vector.scalar_tensor_tensor(
                lu[:], pl[:], rec[:].opt(), bfc_sb[:], ALU.mult, ALU.add
            )
            rmax = st.tile([B, 1], F32, tag="hd", name="rmax")
            nc.vector.tensor_reduce(rmax[:], lu[:], mybir.AxisListType.X,
                                    ALU.max)
            t2 = sb.tile([B, 10], F32, tag="hd2", name="t2")
            nc.vector.tensor_scalar(t2[:], lu[:], rmax[:].opt(), None,
                                    ALU.subtract)
            et = sb.tile([B, 10], F32, tag="hd2", name="et")
            se = st.tile([B, 1], F32, tag="hd", name="se")
            nc.scalar.activation(et[:], t2[:], AF.Exp, accum_out=se[:])
            ls = st.tile([B, 1], F32, tag="hd", name="ls")
            nc.scalar.activation(ls[:], se[:], AF.Ln)
            outv = sb.tile([B, 10], F32, tag="hd2", name="outv")
            nc.vector.tensor_scalar(outv[:], t2[:], ls[:].opt(), None,
                                    ALU.subtract)
            nc.sync.dma_start(out_d.ap(), outv[:])

    nc.compile()
    return nc


_NC = None


def _get_nc():
    global _NC
    if _NC is None:
        _NC = _build()
    return _NC


def _host_prep(inputs):
    """Build the 8 per-core input maps. Core c: sample s=c//4, slice r=c%4."""
    Di = np.ascontiguousarray(np.asarray(inputs["Di"]), np.float32)
    DiA = np.ascontiguousarray(np.asarray(inputs["DiA"]), np.float32)
    inp = np.asarray(inputs["inputs"], np.float32)
    mask = np.asarray(inputs["mask"], np.float32)[:, :, 0]   # [2, 1024]

    def dup(a):  # stack weights for both sample halves on K
        return np.concatenate([a, a], axis=0)

    base = {}
    base["w_in"] = np.asarray(inputs["W_in"]).astype(NP_BF16)
    base["b_in2"] = np.tile(
        np.asarray(inputs["b_in"], np.float32).reshape(C, 1), (2, 1))
    base["w0"] = dup(np.ascontiguousarray(
        np.asarray(inputs["rn_W0"]).transpose(1, 0, 2))).astype(NP_BF16)
    base["w1"] = dup(np.ascontiguousarray(
        np.asarray(inputs["rn_W1"]).transpose(1, 0, 2))).astype(NP_BF16)
    base["b0"] = np.asarray(inputs["rn_b0"]).astype(NP_BF16)[None, :, :]
    base["b1"] = np.asarray(inputs["rn_b1"]).astype(NP_BF16)[None, :, :]
    g0_full = np.tile(np.ascontiguousarray(
        np.asarray(inputs["rn_g0"]).T).astype(np.float32), (2, 1))
    be0_full = np.tile(np.ascontiguousarray(
        np.asarray(inputs["rn_be0"]).T).astype(np.float32), (2, 1))
    g1_full = np.tile(np.ascontiguousarray(
        np.asarray(inputs["rn_g1"]).T).astype(np.float32), (2, 1))
    be1_full = np.tile(np.ascontiguousarray(
        np.asarray(inputs["rn_be1"]).T).astype(np.float32), (2, 1))
    base["sselv"] = np.tile(np.eye(64, dtype=np.float32), (2, 2))
    base["bn2g"] = np.tile(
        np.asarray(inputs["bn2_g"]).astype(np.float32).reshape(C, 1), (2, 1))
    base["bn2b"] = np.tile(
        np.asarray(inputs["bn2_b"]).astype(np.float32).reshape(C, 1), (2, 1))
    base["w2"] = dup(np.asarray(inputs["W2"])).astype(NP_BF16)
    base["b2"] = np.asarray(inputs["b2"]).astype(NP_BF16).reshape(1, C)
    base["wfc"] = np.asarray(inputs["Wfc"]).astype(NP_BF16)
    base["bfc"] = np.broadcast_to(
        np.asarray(inputs["bfc"], np.float32), (B, 10)).copy()
    base["inpT"] = np.ascontiguousarray(inp.transpose(2, 0, 1)).astype(NP_BF16)
    base["maskc"] = np.ascontiguousarray(
        mask.reshape(2, 8, 128).transpose(2, 1, 0)).astype(NP_BF16)
    base["maskrow"] = mask.astype(NP_BF16)

    in_maps = []
    for c in range(NCORES):
        s, r = c // 4, c % 4
        m = dict(base)
        Dr = Di[s].reshape(F, 4, N, 4)          # [p, j, n, jj]
        P4 = Dr[512 * r:512 * (r + 1)]          # [512, 4, 1024, 4]
        DiTg = P4.reshape(512, 4, 8, 128, 4).transpose(2, 4, 3, 1, 0) \
                 .reshape(4096, 2048)           # rows (n8,jj,n'), cols (j,p')
        m["dit"] = np.ascontiguousarray(
            DiTg.reshape(32, 128, 2048).transpose(1, 0, 2)).astype(NP_FP8)
        A = DiA[s].reshape(N, 4, F, 4)          # [n, j, p, jj]
        A4 = A[256 * r:256 * (r + 1)]           # [256, 4, 2048, 4]
        DiATg = A4.reshape(256, 4, 16, 128, 4).transpose(2, 4, 3, 1, 0) \
                  .reshape(8192, 1024)          # rows (pc,jj,p''), cols (j,n')
        m["diat"] = np.ascontiguousarray(
            DiATg.reshape(64, 128, 1024).transpose(1, 0, 2)).astype(NP_FP8)
        # msel folded into gamma/beta: zero the non-owned sample's BN affine
        mselv = np.zeros((128, 1), np.float32)
        mselv[64 * s:64 * (s + 1)] = 1.0
        m["g0"] = g0_full * mselv
        m["be0"] = be0_full * mselv
        m["g1"] = g1_full * mselv
        m["be1"] = be1_full * mselv
        in_maps.append(m)
    return in_maps


def _run(inputs, trace=False, **kw):
    nc = _get_nc()
    in_maps = _host_prep(inputs)
    res = run_bass_kernel_spmd(
        nc, in_maps, core_ids=list(range(NCORES)), trace=trace, **kw
    )
    out = np.asarray(res.results[0]["out"], np.float32).copy()
    return out, res


def kernel(**inputs):
    out, _ = _run(inputs, trace=False)
    return out


# revision 48
# speedup vs baseline: 1.1959x; 1.1959x over previous
"""Trainium2 Bass kernel for nn_DirModel (quaternion Dirac GNN message passing).

Strategy (8 NeuronCores, B=2 samples):
  - 4 cores per sample: core c owns sample s=c//4 and slice r=c%4 of the face
    rows (Di output) / node rows (DiA output).
  - Di/DiA are host-side transposed/permuted/sliced, cast to fp8(e4m3), and
    kept RESIDENT in SBUF (8+8 MB per core) across all 5 blocks.
  - Big matmuls keep the activations stationary (lhsT [128,16] quaternion
    slices) and stream the resident fp8 matrix as moving operand,
    accumulating over the contraction in PSUM. The 4 output groups run
    column-tiled (tile_position=(0,32j)) with each group in its OWN PSUM
    bank, and the accumulation steps are emitted interleaved with the
    produce chunks (4 steps per chunk) so the PE stream is dense with no
    serial tail.
  - Both samples are stacked on the partition axis: states vTB/fTB are
    [128, rows] bf16 with partitions 64*s+c. All elementwise/BN work runs at
    full 128-lane width; the per-node linears contract K=128 (both samples at
    once) with duplicated weights, the non-owned sample zeroed via msel
    premultiplied into gamma/beta host-side.
  - Per block, slices are exchanged with one 8-rank AllGather per direction.
  - BN batch stats piggyback on the elu ops (accum_out); the cross-partition
    combine uses a selector matmul with 1/T pre-folded so it yields E[x],
    E[x^2] directly; rstd via ACT Sqrt + DVE reciprocal. The per-block bias
    row (with the BN shift folded in) is preloaded into PSUM by 1-2 wide K=1
    matmuls per segment (rhs = host-replicated 512-wide bias row) instead of
    one K=1 matmul per chunk.
"""

import numpy as np
import ml_dtypes

import concourse.bass as bass
import concourse.mybir as mybir
import concourse.tile as tile
from concourse import bacc
from concourse.bass_utils import run_bass_kernel_spmd

B, N, F, C = 2, 1024, 2048, 64
NB = 5
NCORES = 8

F32 = mybir.dt.float32
BF16 = mybir.dt.bfloat16
FP8 = mybir.dt.float8e4
NP_BF16 = ml_dtypes.bfloat16
NP_FP8 = ml_dtypes.float8_e4m3
AF = mybir.ActivationFunctionType
ALU = mybir.AluOpType
RG = [list(range(NCORES))]


def _build():
    nc = bacc.Bacc(
        "TRN2",
        target_bir_lowering=False,
        debug=False,
        enable_asserts=False,
        num_devices=NCORES,
    )

    # ---------------- DRAM I/O ----------------
    dit_d = nc.dram_tensor("dit", [128, 32, 2048], FP8, kind="ExternalInput")
    diat_d = nc.dram_tensor("diat", [128, 64, 1024], FP8, kind="ExternalInput")
    inpT_d = nc.dram_tensor("inpT", [3, B, N], BF16, kind="ExternalInput")
    w_in_d = nc.dram_tensor("w_in", [3, C], BF16, kind="ExternalInput")
    b_in2_d = nc.dram_tensor("b_in2", [128, 1], F32, kind="ExternalInput")
    w0_d = nc.dram_tensor("w0", [128, NB, C], BF16, kind="ExternalInput")
    w1_d = nc.dram_tensor("w1", [128, NB, C], BF16, kind="ExternalInput")
    b0_d = nc.dram_tensor("b0", [1, NB, C], BF16, kind="ExternalInput")
    b1_d = nc.dram_tensor("b1", [1, NB, C], BF16, kind="ExternalInput")
    g0_d = nc.dram_tensor("g0", [128, NB], F32, kind="ExternalInput")
    be0_d = nc.dram_tensor("be0", [128, NB], F32, kind="ExternalInput")
    g1_d = nc.dram_tensor("g1", [128, NB], F32, kind="ExternalInput")
    be1_d = nc.dram_tensor("be1", [128, NB], F32, kind="ExternalInput")
    sselv_d = nc.dram_tensor("sselv", [128, 128], F32, kind="ExternalInput")
    bn2g_d = nc.dram_tensor("bn2g", [128, 1], F32, kind="ExternalInput")
    bn2b_d = nc.dram_tensor("bn2b", [128, 1], F32, kind="ExternalInput")
    w2_d = nc.dram_tensor("w2", [128, C], BF16, kind="ExternalInput")
    b2_d = nc.dram_tensor("b2", [1, C], BF16, kind="ExternalInput")
    maskc_d = nc.dram_tensor("maskc", [128, 8, B], BF16, kind="ExternalInput")
    maskrow_d = nc.dram_tensor("maskrow", [B, N], BF16, kind="ExternalInput")
    wfc_d = nc.dram_tensor("wfc", [C, 10], BF16, kind="ExternalInput")
    bfc_d = nc.dram_tensor("bfc", [B, 10], F32, kind="ExternalInput")
    out_d = nc.dram_tensor("out", [B, 10], F32, kind="ExternalOutput")

    with tile.TileContext(nc) as tc:
        with (
            tc.tile_pool(name="res", bufs=1) as res,
            tc.tile_pool(name="sb", bufs=2) as sb,
            tc.tile_pool(name="sc", bufs=2) as sc,
            tc.tile_pool(name="st", bufs=4) as st,
            tc.tile_pool(name="pacc", bufs=1, space="PSUM") as pacc,
            tc.tile_pool(name="px", bufs=1, space="PSUM") as px,
            tc.tile_pool(name="pm", bufs=1, space="PSUM") as pm,
            tc.tile_pool(name="dram", bufs=2, space="DRAM") as dram,
        ):
            # ------------- resident loads (spread across the 3 DGE queues;
            # dit chunks land first so block-0's Di matmul can ride them) ----
            small_engs = (nc.sync, nc.scalar)
            _sl = [0]

            def load(name, shape, dtype, src):
                t = res.tile(shape, dtype, name=name)
                small_engs[_sl[0] % 2].dma_start(t[:], src.ap())
                _sl[0] += 1
                return t

            inpT_sb = load("inpT_sb", [3, B, N], BF16, inpT_d)
            w_in_sb = load("w_in_sb", [3, C], BF16, w_in_d)
            b_in2_sb = load("b_in2_sb", [128, 1], F32, b_in2_d)
            w0_sb = load("w0_sb", [128, NB, C], BF16, w0_d)
            w1_sb = load("w1_sb", [128, NB, C], BF16, w1_d)
            b0_sb = load("b0_sb", [1, NB, C], BF16, b0_d)
            b1_sb = load("b1_sb", [1, NB, C], BF16, b1_d)
            g0_sb = load("g0_sb", [128, NB], F32, g0_d)
            be0_sb = load("be0_sb", [128, NB], F32, be0_d)
            g1_sb = load("g1_sb", [128, NB], F32, g1_d)
            be1_sb = load("be1_sb", [128, NB], F32, be1_d)
            sselv_sb = load("sselv_sb", [128, 128], F32, sselv_d)
            bn2g_sb = load("bn2g_sb", [128, 1], F32, bn2g_d)
            bn2b_sb = load("bn2b_sb", [128, 1], F32, bn2b_d)
            w2_sb = load("w2_sb", [128, C], BF16, w2_d)
            b2_sb = load("b2_sb", [1, C], BF16, b2_d)
            maskc_sb = load("maskc_sb", [128, 8, B], BF16, maskc_d)
            maskrow_sb = load("maskrow_sb", [B, N], BF16, maskrow_d)
            wfc_sb = load("wfc_sb", [C, 10], BF16, wfc_d)
            bfc_sb = load("bfc_sb", [B, 10], F32, bfc_d)

            # big resident operators: all dit chunks first (block 0 needs
            # them soonest), then diat, round-robin across the 3 DGE queues.
            big_engs = (nc.gpsimd, nc.sync, nc.scalar)
            dit_v = dit_d.ap().rearrange("p (a k) n -> p a k n", a=8)
            dit_cs = []
            for a in range(8):
                t = res.tile([128, 4, 2048], FP8, name=f"dit{a}")
                big_engs[a % 3].dma_start(t[:], dit_v[:, a])
                dit_cs.append(t)
            diat_v = diat_d.ap().rearrange("p (a k) n -> p a k n", a=8)
            diat_cs = []
            for a in range(8):
                t = res.tile([128, 8, 1024], FP8, name=f"diat{a}")
                big_engs[a % 3].dma_start(t[:], diat_v[:, a])
                diat_cs.append(t)

            ones_bf = res.tile([1, 128], BF16)
            nc.vector.memset(ones_bf[:], 1.0)

            # ------------- state: both samples stacked on partitions ------
            vTB = res.tile([128, N], BF16)   # partition 64*s + c
            fTB = res.tile([128, F], BF16)

            # four per-column-group PSUM accumulator banks, shared between
            # the Di (cols 0:512) and DiA (cols 0:256) phases.
            bigp = [pacc.tile([128, 512], F32, tag=f"bg{j}", name=f"bg{j}")
                    for j in range(4)]
            # bias-preloaded produce accumulators; chunks alternate between
            # the two tiles so dependency tracking (tile-granular) doesn't
            # serialize chunk t+1's matmul behind chunk t's elu reads.
            pxt = [px.tile([128, 512], F32, tag=f"px{p}", name=f"px{p}")
                   for p in range(2)]

            # initial v = inputs @ W_in + b_in (borrows pxt)
            for h in range(2):
                for s in range(B):
                    nc.tensor.matmul(
                        pxt[h][64 * s:64 * (s + 1), :],
                        w_in_sb[:],
                        inpT_sb[:, s, 512 * h:512 * (h + 1)],
                        start=True, stop=True,
                        tile_position=(0, 64 * s),
                    )
                nc.vector.tensor_scalar(
                    vTB[:, 512 * h:512 * (h + 1)],
                    pxt[h][:],
                    b_in2_sb[:].opt(), None, ALU.add,
                )
            nc.vector.memset(fTB[:], 0.0)

            def elu_stats(src, R, nm):
                """elu(src)->[128,R] bf16 + per-half stat tiles [128,2]
                (sum, sumsq) so the selector matmul of half 0 can overlap
                half 1's stats."""
                ev = sc.tile([128, R], BF16, tag="evT", name=f"ev{nm}")
                sqd = sc.tile([128, R], BF16, tag="sqT", name=f"sq{nm}")
                stats = []
                H = R // 2
                for h in range(2):
                    cs = slice(H * h, H * (h + 1))
                    e = sc.tile([128, H], BF16, tag=f"eT{h}", name=f"e{nm}{h}")
                    nc.scalar.activation(e[:], src[:, cs], AF.Exp)
                    nc.vector.tensor_scalar(
                        e[:], e[:], -1.0, 0.0, ALU.add, ALU.min)
                    sth = st.tile([128, 2], F32, tag=f"sth{h}",
                                  name=f"sth{nm}{h}")
                    nc.vector.scalar_tensor_tensor(
                        ev[:, cs], src[:, cs], 0.0, e[:], ALU.max, ALU.add,
                        accum_out=sth[:, 0:1],
                    )
                    nc.scalar.activation(
                        sqd[:, cs], ev[:, cs], AF.Square,
                        accum_out=sth[:, 1:2])
                    stats.append(sth)
                return ev, stats

            def rsqrt_chain(pst, g_ap, be_ap, nm):
                """pst [128,2] SBUF = [E[x], E[x^2]] -> (scale f32, shneg
                bf16). Bit-trick + 1 Newton step keeps ACT tables on EXP."""
                m2 = st.tile([128, 1], F32, tag="bns2", name=f"m2{nm}")
                nc.vector.tensor_mul(m2[:], pst[:, 0:1], pst[:, 0:1])
                varp = st.tile([128, 1], F32, tag="bns3", name=f"vp{nm}")
                nc.vector.tensor_tensor(varp[:], pst[:, 1:2], m2[:],
                                        ALU.subtract)
                iv = st.tile([128, 1], mybir.dt.int32, tag="bnsi",
                             name=f"iv{nm}")
                nc.vector.tensor_scalar(
                    iv[:], varp[:].bitcast(mybir.dt.int32), 1, None,
                    ALU.arith_shift_right,
                )
                nc.vector.tensor_scalar(
                    iv[:], iv[:], -1, 0x5F3759DF, ALU.mult, ALU.add
                )
                rstd = st.tile([128, 1], F32, tag="bns4", name=f"rstd{nm}")
                nc.vector.tensor_copy(rstd[:], iv[:].bitcast(F32))
                nt = st.tile([128, 1], F32, tag="bns5", name=f"nt{nm}")
                nc.vector.tensor_mul(nt[:], rstd[:], rstd[:])
                nc.vector.tensor_mul(nt[:], nt[:], varp[:])
                nc.vector.tensor_scalar(
                    nt[:], nt[:], -0.5, 1.5, ALU.mult, ALU.add
                )
                nc.vector.tensor_mul(rstd[:], rstd[:], nt[:])
                scale = st.tile([128, 1], F32, tag="selsc", name=f"sc{nm}")
                nc.vector.tensor_mul(scale[:], rstd[:], g_ap)
                shneg = st.tile([128, 1], BF16, tag="selsh", name=f"sh{nm}")
                nc.vector.scalar_tensor_tensor(
                    shneg[:], pst[:, 0:1], scale[:].opt(), be_ap,
                    ALU.mult, ALU.subtract,
                )
                return scale, shneg

            def bn_fold(ps, stats, ssel, T, g_ap, be_ap, w_ap, b_ap, nm):
                """-> (Wp bf16 [128,C], brow bf16 [1,C]).

                Per-half accumulating selector matmuls (half 0's can run
                while half 1's stats finish). The 1/T scaling happens on
                the DVE during the PSUM->SBUF copy (full f32), not in the
                PE fp32r matmul."""
                for h in range(2):
                    nc.tensor.matmul(ps[:, 0:2], ssel, stats[h][:],
                                     start=(h == 0), stop=(h == 1))
                pst = st.tile([128, 2], F32, tag="pstc", name=f"pst{nm}")
                nc.vector.tensor_scalar(pst[:], ps[:, 0:2], 1.0 / T, None,
                                        ALU.mult)
                scale, shneg = rsqrt_chain(pst, g_ap, be_ap, nm)
                Wp = sc.tile([128, C], BF16, tag="wp", name=f"wp{nm}")
                nc.vector.tensor_scalar(
                    Wp[:], w_ap, scale[:].opt(), None, ALU.mult)
                psr = pm.tile([1, C], F32, tag="prow", name=f"psr{nm}")
                nc.tensor.matmul(psr[:], shneg[:], w_ap, start=True,
                                 stop=True)
                brow = sc.tile([1, C], BF16, tag="brow", name=f"brow{nm}")
                nc.vector.tensor_tensor(brow[:], b_ap, psr[:], ALU.subtract)
                return Wp, brow

            def stage_gather(nsz, nm):
                """CAST the 4 group banks -> compact fp8 stage, 1 DMA out,
                AllGather, and return the SBUF view of the gathered data."""
                stg = sb.tile([128, nsz], FP8, tag="stg", name=f"stg{nm}")
                for j in range(4):
                    if j % 2 == 0:
                        nc.vector.tensor_copy(
                            stg[32 * j:32 * j + 16, :],
                            bigp[j][32 * j:32 * j + 16, 0:nsz])
                    else:
                        nc.scalar.copy(
                            stg[32 * j:32 * j + 16, :],
                            bigp[j][32 * j:32 * j + 16, 0:nsz])
                agin = dram.tile([C, nsz], FP8, tag="agin", name=f"agin{nm}")
                nc.sync.dma_start(
                    agin[:].rearrange("(a b) n -> a b n", a=4),
                    stg[:].rearrange("(a b) n -> a b n", a=4)[:, 0:16, :],
                )
                agout = dram.tile([NCORES * C, nsz], FP8, tag="agout",
                                  name=f"agout{nm}", addr_space="Shared")
                nc.gpsimd.collective_compute(
                    "AllGather", ALU.bypass, replica_groups=RG,
                    ins=[agin.opt()], outs=[agout.opt()],
                )
                agv = agout[:].rearrange("(s r c) n -> s c r n", s=2, r=4)
                gB = sb.tile([128, 4 * nsz], FP8, tag="gB", name=f"gB{nm}")
                # one DMA per (sample, r-slice): 64 descriptors each, so the
                # earliest quarter's state add can start ~2us sooner.
                g_engs = (nc.sync, nc.scalar, nc.gpsimd)
                k = 0
                for r in range(4):
                    for s in range(B):
                        g_engs[k % 3].dma_start(
                            gB[C * s:C * (s + 1),
                               r * nsz:(r + 1) * nsz],
                            agv[s][:, r, :])
                        k += 1
                return gB

            def accum_state_q(stT, gB, R):
                """state += gathered, in quarters chained to the per-r
                gather DMAs."""
                Q = R // 4
                for q in range(4):
                    cs = slice(Q * q, Q * (q + 1))
                    nc.vector.tensor_tensor(
                        stT[:, cs], stT[:, cs], gB[:, cs], ALU.add)

            def warm_pe(ps, gB, ncols, nm):
                """~16 tiny matmuls anchored on the gathered payload keep
                the PE's HAM clock un-throttled through the stats phase so
                the real matmul stream starts warm."""
                for k in range(16):
                    nc.tensor.matmul(
                        ps[:, 2:4],
                        gB[:, 128 * (k % (ncols // 128)):
                           128 * (k % (ncols // 128)) + 128],
                        gB[:, 0:2],
                        start=True, stop=True,
                    )



            def produce_chunk(ev_ap, t, Wp, out_dtype, nm):
                """row[t] [128, C] = elu(ev[:,128t:].T @ W' + brow).

                Bias was preloaded into pxt by the wide K=1 matmuls."""
                psx = pxt[t % 2][:, 64 * (t // 2):64 * (t // 2) + 64]
                nc.tensor.matmul(psx, ev_ap, Wp[:], start=False, stop=True)
                e2 = sc.tile([128, C], BF16, tag="e2", name=f"e2{nm}{t}")
                nc.scalar.activation(e2[:], psx, AF.Exp)
                nc.vector.tensor_scalar(e2[:], e2[:], -1.0, 0.0,
                                        ALU.add, ALU.min)
                row = sc.tile([128, C], out_dtype, tag=f"rowt{t}",
                              name=f"row{nm}{t}")
                nc.vector.scalar_tensor_tensor(
                    row[:], psx, 0.0, e2[:], ALU.max, ALU.add
                )
                return row

            def preload_bias(brow, nchunks):
                rhs = brow[:].unsqueeze(1).broadcast_to([1, 8, C])
                for h in range(2):
                    nc.tensor.matmul(
                        pxt[h][:], ones_bf[:],
                        rhs, start=True, stop=False,
                    )

            LAG = 2

            def di_steps(t, xr_t, i):
                for jj in range(4):
                    kk = 4 * t + jj
                    for j in range(4):
                        nc.tensor.matmul(
                            bigp[j][32 * j:32 * j + 16, :],
                            xr_t[:, 16 * jj:16 * (jj + 1)],
                            dit_cs[t][:, jj, 512 * j:512 * (j + 1)],
                            start=(kk == 0), stop=(kk == 31),
                            tile_position=(0, 32 * j),
                        )

            def dia_steps(t, yr_t, i):
                for m in range(4):
                    kk = 4 * t + m
                    for j in range(4):
                        nc.tensor.matmul(
                            bigp[j][32 * j:32 * j + 16, 0:256],
                            yr_t[:, 16 * m:16 * (m + 1)],
                            diat_cs[kk // 8][:, kk % 8,
                                             256 * j:256 * (j + 1)],
                            start=(kk == 0), stop=(kk == 63),
                            tile_position=(0, 32 * j),
                        )

            ps_next = None
            for i in range(NB):
                # ======== v side: x = elu(conv(elu(v))), f += Di @ x ======
                ev, vstats = elu_stats(vTB[:], N, f"v{i}")
                if ps_next is None:
                    ps_next = pm.tile([128, 4], F32, tag="pstat",
                                      name=f"psv{i}")
                Wp0, brow0 = bn_fold(
                    ps_next, vstats, sselv_sb[:], float(B * N),
                    g0_sb[:, i:i + 1].opt(),
                    be0_sb[:, i:i + 1].opt(), w0_sb[:, i, :],
                    b0_sb[:, i, :], f"v{i}",
                )
                preload_bias(brow0, 8)
                xrs = {}
                for t in range(8 + LAG):
                    if t < 8:
                        xrs[t] = produce_chunk(
                            ev[:, 128 * t:128 * (t + 1)], t, Wp0, FP8,
                            f"x{i}")
                    if t >= LAG:
                        di_steps(t - LAG, xrs[t - LAG], i)
                gfB = stage_gather(512, f"f{i}")
                psf = pm.tile([128, 4], F32, tag="pstat", name=f"psf{i}")
                warm_pe(psf, gfB, 2048, f"f{i}")
                accum_state_q(fTB, gfB, F)

                # ======== f side: y = elu(conv(elu(f))), v += DiA @ y =====
                ef, fstats = elu_stats(fTB[:], F, f"f{i}")
                Wp1, brow1 = bn_fold(
                    psf, fstats, sselv_sb[:], float(B * F),
                    g1_sb[:, i:i + 1].opt(),
                    be1_sb[:, i:i + 1].opt(), w1_sb[:, i, :],
                    b1_sb[:, i, :], f"f{i}",
                )
                preload_bias(brow1, 16)
                yrs = {}
                for t in range(16 + LAG):
                    if t < 16:
                        yrs[t] = produce_chunk(
                            ef[:, 128 * t:128 * (t + 1)], t, Wp1, FP8,
                            f"y{i}")
                    if t >= LAG:
                        dia_steps(t - LAG, yrs[t - LAG], i)
                gvB = stage_gather(256, f"v{i}")
                ps_next = pm.tile([128, 4], F32, tag="pstat",
                                  name=f"psv{i + 1}")
                warm_pe(ps_next, gvB, 1024, f"v{i + 1}")
                accum_state_q(vTB, gvB, N)

            # ======== head (both samples, no selection) ========
            hev, hstats = elu_stats(vTB[:], N, "h")
            ps_h = ps_next
            for h in range(2):
                nc.tensor.matmul(ps_h[:, 0:2], sselv_sb[:], hstats[h][:],
                                 start=(h == 0), stop=(h == 1))
            pst_h = st.tile([128, 2], F32, tag="pstc", name="pst_h")
            nc.vector.tensor_scalar(pst_h[:], ps_h[:, 0:2],
                                    1.0 / float(B * N), None, ALU.mult)
            hscale, hshneg = rsqrt_chain(pst_h, bn2g_sb[:], bn2b_sb[:], "h")
            # head contraction is per-sample (K=64): fold with K=64 shift
            Wp2 = sc.tile([128, C], BF16, tag="wp", name="wp_h")
            nc.vector.tensor_scalar(
                Wp2[:], w2_sb[:], hscale[:].opt(), None, ALU.mult)
            psr_h = pm.tile([1, C], F32, tag="prow", name="psr_h")
            nc.tensor.matmul(psr_h[:], hshneg[0:C, :], w2_sb[0:C, :],
                             start=True, stop=True)
            brow_h = sc.tile([1, C], BF16, tag="brow", name="brow_h")
            nc.vector.tensor_tensor(brow_h[:], b2_sb[:], psr_h[:],
                                    ALU.subtract)
            brow_h_b = brow_h[:].unsqueeze(1).broadcast_to([1, 8, C])
            pooled = sb.tile([C, B], BF16, tag="pooled")
            for hb in range(2):
                nc.tensor.matmul(
                    pxt[hb][:], ones_bf[:], brow_h_b,
                    start=True, stop=False,
                )
            pp_both = pm.tile([C, B], F32, tag="prow", name="pp_both")
            hrows = {}
            for g in range(16 + LAG):
                if g < 16:
                    s, t = divmod(g, 8)
                    psx = pxt[g % 2][:, 64 * (g // 2):64 * (g // 2) + 64]
                    nc.tensor.matmul(
                        psx, hev[64 * s:64 * (s + 1),
                                 128 * t:128 * (t + 1)],
                        Wp2[64 * s:64 * (s + 1), :],
                        start=False, stop=True,
                    )
                    e2 = sc.tile([128, C], BF16, tag="e2", name=f"e2h{g}")
                    nc.scalar.activation(e2[:], psx, AF.Exp)
                    nc.vector.tensor_scalar(e2[:], e2[:], -1.0, 0.0,
                                            ALU.add, ALU.min)
                    row = sc.tile([128, C], BF16, tag=f"rowt{g % 8}",
                                  name=f"rowh{g}")
                    nc.vector.scalar_tensor_tensor(
                        row[:], psx, 0.0, e2[:], ALU.max, ALU.add
                    )
                    hrows[g] = row
                if g >= LAG:
                    s2, t2 = divmod(g - LAG, 8)
                    nc.tensor.matmul(
                        pp_both[:, s2:s2 + 1], hrows[g - LAG][:],
                        maskc_sb[:, t2, s2:s2 + 1],
                        start=(t2 == 0), stop=(t2 == 7),
                    )
            nc.vector.tensor_copy(pooled[:], pp_both[:])
            msum = st.tile([B, 1], F32, tag="hd", name="msum")
            nc.vector.tensor_reduce(
                msum[:], maskrow_sb[:], mybir.AxisListType.X, ALU.add
            )
            rec = st.tile([B, 1], F32, tag="hd", name="rec")
            nc.vector.reciprocal(rec[:], msum[:])
            pl = pm.tile([B, 10], F32, tag="pstat", name="pl")
            nc.tensor.matmul(pl[:], pooled[:], wfc_sb[:], start=True,
                             stop=True)
            lu = sb.tile([B, 10], F32, tag="hd2", name="lu")
            nc.vector.scalar_tensor_tensor(
                lu[:], pl[:], rec[:].opt(), bfc_sb[:], ALU.mult, ALU.add
            )
            rmax = st.tile([B, 1], F32, tag="hd", name="rmax")
            nc.vector.tensor_reduce(rmax[:], lu[:], mybir.AxisListType.X,
                                    ALU.max)
            t2 = sb.tile([B, 10], F32, tag="hd2", name="t2")
            nc.vector.tensor_scalar(t2[:], lu[:], rmax[:].opt(), None,
                                    ALU.subtract)
            et = sb.tile([B, 10], F32, tag="hd2", name="et")
            se = st.tile([B, 1], F32, tag="hd", name="se")
            nc.scalar.activation(et[:], t2[:], AF.Exp, accum_out=se[:])
            ls = st.tile([B, 1], F32, tag="hd", name="ls")
            nc.scalar.activation(ls[:], se[:], AF.Ln)
            outv = sb.tile([B, 10], F32, tag="hd2", name="outv")
            nc.vector.tensor_scalar(outv[:], t2[:], ls[:].opt(), None,
                                    ALU.subtract)
            nc.sync.dma_start(out_d.ap(), outv[:])

    nc.compile()
    return nc


_NC = None


def _get_nc():
    global _NC
    if _NC is None:
        _NC = _build()
    return _NC


def _host_prep(inputs):
    """Build the 8 per-core input maps. Core c: sample s=c//4, slice r=c%4."""
    Di = np.ascontiguousarray(np.asarray(inputs["Di"]), np.float32)
    DiA = np.ascontiguousarray(np.asarray(inputs["DiA"]), np.float32)
    inp = np.asarray(inputs["inputs"], np.float32)
    mask = np.asarray(inputs["mask"], np.float32)[:, :, 0]   # [2, 1024]

    def dup(a):  # stack weights for both sample halves on K
        return np.concatenate([a, a], axis=0)

    base = {}
    base["w_in"] = np.asarray(inputs["W_in"]).astype(NP_BF16)
    base["b_in2"] = np.tile(
        np.asarray(inputs["b_in"], np.float32).reshape(C, 1), (2, 1))
    base["w0"] = dup(np.ascontiguousarray(
        np.asarray(inputs["rn_W0"]).transpose(1, 0, 2))).astype(NP_BF16)
    base["w1"] = dup(np.ascontiguousarray(
        np.asarray(inputs["rn_W1"]).transpose(1, 0, 2))).astype(NP_BF16)
    base["b0"] = np.asarray(inputs["rn_b0"]).astype(NP_BF16)[None, :, :]
    base["b1"] = np.asarray(inputs["rn_b1"]).astype(NP_BF16)[None, :, :]
    g0_full = np.tile(np.ascontiguousarray(
        np.asarray(inputs["rn_g0"]).T).astype(np.float32), (2, 1))
    be0_full = np.tile(np.ascontiguousarray(
        np.asarray(inputs["rn_be0"]).T).astype(np.float32), (2, 1))
    g1_full = np.tile(np.ascontiguousarray(
        np.asarray(inputs["rn_g1"]).T).astype(np.float32), (2, 1))
    be1_full = np.tile(np.ascontiguousarray(
        np.asarray(inputs["rn_be1"]).T).astype(np.float32), (2, 1))
    base["sselv"] = np.tile(np.eye(64, dtype=np.float32), (2, 2))
    base["bn2g"] = np.tile(
        np.asarray(inputs["bn2_g"]).astype(np.float32).reshape(C, 1), (2, 1))
    base["bn2b"] = np.tile(
        np.asarray(inputs["bn2_b"]).astype(np.float32).reshape(C, 1), (2, 1))
    base["w2"] = dup(np.asarray(inputs["W2"])).astype(NP_BF16)
    base["b2"] = np.asarray(inputs["b2"]).astype(NP_BF16).reshape(1, C)
    base["wfc"] = np.asarray(inputs["Wfc"]).astype(NP_BF16)
    base["bfc"] = np.broadcast_to(
        np.asarray(inputs["bfc"], np.float32), (B, 10)).copy()
    base["inpT"] = np.ascontiguousarray(inp.transpose(2, 0, 1)).astype(NP_BF16)
    base["maskc"] = np.ascontiguousarray(
        mask.reshape(2, 8, 128).transpose(2, 1, 0)).astype(NP_BF16)
    base["maskrow"] = mask.astype(NP_BF16)

    in_maps = []
    for c in range(NCORES):
        s, r = c // 4, c % 4
        m = dict(base)
        Dr = Di[s].reshape(F, 4, N, 4)          # [p, j, n, jj]
        P4 = Dr[512 * r:512 * (r + 1)]          # [512, 4, 1024, 4]
        DiTg = P4.reshape(512, 4, 8, 128, 4).transpose(2, 4, 3, 1, 0) \
                 .reshape(4096, 2048)           # rows (n8,jj,n'), cols (j,p')
        m["dit"] = np.ascontiguousarray(
            DiTg.reshape(32, 128, 2048).transpose(1, 0, 2)).astype(NP_FP8)
        A = DiA[s].reshape(N, 4, F, 4)          # [n, j, p, jj]
        A4 = A[256 * r:256 * (r + 1)]           # [256, 4, 2048, 4]
        DiATg = A4.reshape(256, 4, 16, 128, 4).transpose(2, 4, 3, 1, 0) \
                  .reshape(8192, 1024)          # rows (pc,jj,p''), cols (j,n')
        m["diat"] = np.ascontiguousarray(
            DiATg.reshape(64, 128, 1024).transpose(1, 0, 2)).astype(NP_FP8)
        # msel folded into gamma/beta: zero the non-owned sample's BN affine
        mselv = np.zeros((128, 1), np.float32)
        mselv[64 * s:64 * (s + 1)] = 1.0
        m["g0"] = g0_full * mselv
        m["be0"] = be0_full * mselv
        m["g1"] = g1_full * mselv
        m["be1"] = be1_full * mselv
        in_maps.append(m)
    return in_maps


def _run(inputs, trace=False, **kw):
    nc = _get_nc()
    in_maps = _host_prep(inputs)
    res = run_bass_kernel_spmd(
        nc, in_maps, core_ids=list(range(NCORES)), trace=trace, **kw
    )
    out = np.asarray(res.results[0]["out"], np.float32).copy()
    return out, res


def kernel(**inputs):
    out, _ = _run(inputs, trace=False)
    return out
